# revision 13
# baseline (speedup 1.0000x reference)
"""DCRNN (2x GCNConv + GRU-over-nodes + Linear) on 8 Trainium2 cores.

Strategy (v2)
-------------
* The normalized adjacency A = D^-1/2 (Adj+I) D^-1/2 is factored as
  dinv[s] * M[s,d] * dinv[d] with M a small-integer matrix (edge
  multiplicity + self loops).  M is shipped as an fp8e4 strip (integers
  are exact in fp8), loaded ONCE into SBUF (15MB resident) and expanded
  to fp16 tile-by-tile on the otherwise-idle Vector/Scalar engines.
  dinv[s] is folded into the dense operand (x resp. XW2); dinv[d] is a
  per-column multiply fused into the PSUM evacuation.
* GCN1 uses associativity: A @ (x @ W1) = (A @ x) @ W1 -- the aggregate
  runs over the 64-wide x (4x less PE work than aggregating 256-wide
  XW1), then a tiny K=64 matmul applies W1.
* GCN2: each core computes h2 for its 1250 rows + 64-row left halo from
  the gathered XW2.  The XW2 AllGather is split into 3 src-group
  collectives (2/4/4 tiles per core) so the GCN2 K-loop can start on
  group A while groups B/C are still in flight; the core's own rows are
  processed first (no gather dependency at all).
* GRU over the 10000-node sequence: K Jacobi sweeps; gates from the
  previous sweep's h, then the exact affine scan (tensor_tensor_scan).
  The 64-row halo makes cores independent (contraction ~0.74/step).
* Final Linear on the node shard; host concatenates the 8 shards.
"""

import numpy as np

NUM_NODES = 10000
IN_FEAT = 64
HID = 256
OUT = 3
CORES = 8
ROWS = NUM_NODES // CORES          # 1250
HALO = 64
L = ROWS + HALO                    # 1314 local sequence length
SWEEPS = 7
KP = 128
# global strip tiles: grpA 16 (2/core) + grpB 32 (4/core) + grpC 32 (4/core,
# last tile of each core zero-padded 98->128) = 80; append (own rows) 10
MT = 80
NLOC = 10
MT2 = MT + NLOC                    # 90 K-tiles
GRP = (MT2 + 3) // 4               # 23 interleave groups of 4 K-tiles
# src-group split (in 128-row tiles per core)
GA_T, GB_T, GC_T = 2, 4, 4         # tiles per core in groups A, B, C
GA_R, GB_R = 256, 512              # real rows per core in A, B
GC_R = ROWS - GA_R - GB_R          # 482 real rows in C (padded to 512)

_CACHE = {}


def _chunks(total, step=512):
    return [(c, min(c + step, total)) for c in range(0, total, step)]


def build_program():
    import concourse.bass as bass
    import concourse.mybir as mybir
    import concourse.tile as tile
    from concourse import bacc

    f16 = mybir.dt.float16
    f8 = mybir.dt.float8e4
    f32 = mybir.dt.float32
    AF = mybir.ActivationFunctionType
    ALU = mybir.AluOpType

    nc = bacc.Bacc("TRN2", num_devices=CORES)

    # ---- inputs ----
    # fp8 integer strip, 4-way K-tile interleaved: row (g*128+p), col
    # (j*L+c) holds M[src-of-tile(4g+j) p, window col c].
    a2t_d = nc.dram_tensor("a2t", [GRP * KP, 4 * L], f8, kind="ExternalInput")
    # xd = x * dinv[s], strip row order, [128 part, MT2*64]
    xd_d = nc.dram_tensor("xd", [KP, MT2 * IN_FEAT], f16, kind="ExternalInput")
    w1_d = nc.dram_tensor("w1", [IN_FEAT, HID], f16, kind="ExternalInput")
    w2_d = nc.dram_tensor("w2", [HID, HID], f16, kind="ExternalInput")
    wiht_d = nc.dram_tensor("wiht", [HID, 3 * HID], f16, kind="ExternalInput")
    whht_d = nc.dram_tensor("whht", [HID, 3 * HID], f16, kind="ExternalInput")
    fcwt_d = nc.dram_tensor("fcwt", [HID, OUT], f16, kind="ExternalInput")
    ident_d = nc.dram_tensor("ident", [KP, KP], f16, kind="ExternalInput")
    b1c_d = nc.dram_tensor("b1c", [KP, 2], f32, kind="ExternalInput")
    b2c_d = nc.dram_tensor("b2c", [KP, 2], f32, kind="ExternalInput")
    gib_d = nc.dram_tensor("gib", [KP, 6], f32, kind="ExternalInput")
    bhn_d = nc.dram_tensor("bhn", [KP, 2], f32, kind="ExternalInput")
    fcb_d = nc.dram_tensor("fcb", [KP, 1], f32, kind="ExternalInput")
    patch_d = nc.dram_tensor("patch", [KP, 12], f32, kind="ExternalInput")
    dinv1_d = nc.dram_tensor("dinv1", [IN_FEAT, ROWS], f16, kind="ExternalInput")
    dinv2_d = nc.dram_tensor("dinv2", [KP, L], f16, kind="ExternalInput")
    dinvl_d = nc.dram_tensor("dinvl", [KP, NLOC], f32, kind="ExternalInput")
    out_d = nc.dram_tensor("out_t", [OUT, ROWS], f32, kind="ExternalOutput")

    with tile.TileContext(nc) as tc:
        with (
            tc.tile_pool(name="const", bufs=1) as cpool,
            tc.tile_pool(name="big", bufs=1) as big,
            tc.tile_pool(name="tmp", bufs=3) as tpool,
            tc.tile_pool(name="psxw", bufs=2, space="PSUM") as psxw,
            tc.tile_pool(name="dram", bufs=1, space="DRAM") as dpool,
        ):
            sp_cm = tc.tile_pool(name="strip", bufs=1)
            spool = sp_cm.__enter__()
            st_cm = tc.tile_pool(name="stage", bufs=3)
            stpool = st_cm.__enter__()
            # ---- load constants ----
            w1_sb = cpool.tile([IN_FEAT, HID], f16)
            w2_sb = cpool.tile([KP, 2, HID], f16)
            wiht_sb = cpool.tile([KP, 2, 3 * HID], f16)
            whht_sb = cpool.tile([KP, 2, 3 * HID], f16)
            fcwt_sb = cpool.tile([KP, 2, OUT], f16)
            ident_sb = cpool.tile([KP, KP], f16)
            b1c_sb = cpool.tile([KP, 2], f32)
            b2c_sb = cpool.tile([KP, 2], f32)
            gib_sb = cpool.tile([KP, 6], f32)
            bhn_sb = cpool.tile([KP, 2], f32)
            fcb_sb = cpool.tile([KP, 1], f32)
            patch_sb = cpool.tile([KP, 12], f32)
            dinv1_sb = cpool.tile([IN_FEAT, ROWS], f16)
            dinv2_sb = cpool.tile([KP, L], f16)
            dinvl_sb = cpool.tile([KP, NLOC], f32)

            nc.sync.dma_start(ident_sb[:], ident_d[:])
            nc.sync.dma_start(w1_sb[:], w1_d[:])
            for k in range(2):
                nc.sync.dma_start(w2_sb[:, k, :], w2_d[k * KP:(k + 1) * KP, :])
                nc.sync.dma_start(wiht_sb[:, k, :], wiht_d[k * KP:(k + 1) * KP, :])
                nc.sync.dma_start(whht_sb[:, k, :], whht_d[k * KP:(k + 1) * KP, :])
                nc.sync.dma_start(fcwt_sb[:, k, :], fcwt_d[k * KP:(k + 1) * KP, :])
            nc.sync.dma_start(b1c_sb[:], b1c_d[:])
            nc.sync.dma_start(b2c_sb[:], b2c_d[:])
            nc.sync.dma_start(gib_sb[:], gib_d[:])
            nc.sync.dma_start(bhn_sb[:], bhn_d[:])
            nc.sync.dma_start(fcb_sb[:], fcb_d[:])
            nc.sync.dma_start(patch_sb[:], patch_d[:])
            nc.sync.dma_start(dinv1_sb[:], dinv1_d[:])
            nc.sync.dma_start(dinv2_sb[:], dinv2_d[:])
            nc.sync.dma_start(dinvl_sb[:], dinvl_d[:])

            # ---- xd first (GCN1's stationary operand), then the fp8 strip ----
            xd_cm = tc.tile_pool(name="xdp", bufs=1)
            xdp = xd_cm.__enter__()
            xd_sb = xdp.tile([KP, MT2, IN_FEAT], f16)
            nc.scalar.dma_start(xd_sb[:], xd_d[:])
            strip_sb = spool.tile([KP, GRP, 4 * L], f8)
            for g in range(GRP):
                eng = nc.sync if g % 2 == 0 else nc.scalar
                eng.dma_start(strip_sb[:, g, :], a2t_d[g * KP:(g + 1) * KP, :])

            psG_cm = tc.tile_pool(name="psG", bufs=1, space="PSUM")
            psG = psG_cm.__enter__()

            # tiny AllGather to absorb the first-collective ncfw setup cost
            ccw_in = dpool.tile([CORES, 64], f16)
            ccw_out = dpool.tile([CORES * CORES, 64], f16, addr_space="Shared")
            nc.sync.dma_start(ccw_in[0:KP // 16, :], ident_sb[0:8, 0:64])
            nc.gpsimd.collective_compute(
                "AllGather", mybir.AluOpType.bypass,
                replica_groups=[list(range(CORES))],
                ins=[ccw_in.opt()], outs=[ccw_out.opt()])

            # PE warm-up burst so the HAM clock-gate opens before GCN1
            for i in range(40):
                psd = psxw.tile([KP, 512], f32, tag="xwps", name=f"warm_{i}")
                nc.tensor.matmul(psd[:, :KP], ident_sb[:], ident_sb[:],
                                 start=True, stop=True)

            # ---- GCN1: agg1 = (M^T @ xd) on own cols, then h1 = relu(...) ----
            chg1 = _chunks(ROWS)
            ps1 = [psG.tile([KP, 512], f32, tag=f"G{ci}", name=f"ps1_{ci}")
                   for ci in range(3)]
            for k in range(MT2):
                g, j = k // 4, k % 4
                st = stpool.tile([KP, L], f16, tag="st")
                if k % 4 == 1:
                    nc.scalar.activation(st[:, :ROWS],
                                         strip_sb[:, g, j * L + HALO:(j + 1) * L],
                                         AF.Copy)
                elif k % 4 == 3:
                    nc.gpsimd.tensor_copy(st[:, :ROWS],
                                          strip_sb[:, g, j * L + HALO:(j + 1) * L])
                else:
                    nc.vector.tensor_copy(st[:, :ROWS],
                                          strip_sb[:, g, j * L + HALO:(j + 1) * L])
                for ci, (c0, c1) in enumerate(chg1):
                    nc.tensor.matmul(ps1[ci][:IN_FEAT, :c1 - c0],
                                     xd_sb[:, k, :], st[:, c0:c1],
                                     start=(k == 0), stop=(k == MT2 - 1))
            # agg1_sb = ps1 * dinv[d] (column scale)
            agg1_sb = big.tile([IN_FEAT, ROWS], f16)
            for ci, (c0, c1) in enumerate(chg1):
                nc.vector.tensor_tensor(agg1_sb[:, c0:c1],
                                        ps1[ci][:IN_FEAT, :c1 - c0],
                                        dinv1_sb[:, c0:c1], ALU.mult)
            # h1T = relu(W1h^T @ agg1 + b1)
            h1t_sb = big.tile([KP, 2, ROWS], f16)
            for mm in range(2):
                for ci, (c0, c1) in enumerate(chg1):
                    psw = psxw.tile([KP, 512], f32, tag="xwps")
                    nc.tensor.matmul(psw[:, :c1 - c0],
                                     w1_sb[:, mm * KP:(mm + 1) * KP],
                                     agg1_sb[:, c0:c1], start=True, stop=True)
                    nc.scalar.activation(h1t_sb[:, mm, c0:c1],
                                         psw[:, :c1 - c0], AF.Relu,
                                         bias=b1c_sb[:, mm:mm + 1])
            xd_cm.__exit__(None, None, None)

            # ---- XW2 shard = (h1 @ W2) * dinv[s]  (natural layout) ----
            xw2l_sb = big.tile([KP, NLOC, HID], f16)
            for j in range(NLOC):
                rw = min(KP, ROWS - j * KP)
                ps = psxw.tile([KP, 512], f32, tag="xwps")
                for k in range(2):
                    nc.tensor.matmul(ps[:rw, :HID],
                                     h1t_sb[:, k, j * KP:j * KP + rw],
                                     w2_sb[:, k, :],
                                     start=(k == 0), stop=(k == 1))
                nc.scalar.activation(xw2l_sb[:rw, j, :], ps[:rw, :HID],
                                     AF.Copy, scale=dinvl_sb[:rw, j:j + 1])

            # ---- three src-group AllGathers of XW2D ----
            zer_sb = cpool.tile([32, HID], f16)
            nc.vector.memset(zer_sb[:], 0.0)
            bounce_a = dpool.tile([GA_T * KP, HID], f16)
            bounce_b = dpool.tile([GB_T * KP, HID], f16)
            bounce_c = dpool.tile([GC_T * KP, HID], f16)
            gath_a = dpool.tile([CORES * GA_T * KP, HID], f16, addr_space="Shared")
            gath_b = dpool.tile([CORES * GB_T * KP, HID], f16, addr_space="Shared")
            gath_c = dpool.tile([CORES * GC_T * KP, HID], f16, addr_space="Shared")
            for j in range(GA_T):
                nc.sync.dma_start(bounce_a[j * KP:(j + 1) * KP, :],
                                  xw2l_sb[:, j, :])
            for j in range(GB_T):
                nc.sync.dma_start(bounce_b[j * KP:(j + 1) * KP, :],
                                  xw2l_sb[:, GA_T + j, :])
            for j in range(GC_T):
                jj = GA_T + GB_T + j
                rw = min(KP, ROWS - jj * KP)
                nc.sync.dma_start(bounce_c[j * KP:j * KP + rw, :],
                                  xw2l_sb[:rw, jj, :])
                if rw < KP:
                    nc.sync.dma_start(bounce_c[j * KP + rw:(j + 1) * KP, :],
                                      zer_sb[:KP - rw, :])
            for bb, gg in ((bounce_a, gath_a), (bounce_b, gath_b),
                           (bounce_c, gath_c)):
                nc.gpsimd.collective_compute(
                    "AllGather", mybir.AluOpType.bypass,
                    replica_groups=[list(range(CORES))],
                    ins=[bb.opt()], outs=[gg.opt()])

            # ---- GCN2 over the extended (halo) shard ----
            chg2 = _chunks(L)
            ps2 = [[psG.tile([KP, 512], f32,
                             tag=f"G{mm * 3 + ci}", name=f"ps2_{mm}_{ci}")
                    for ci in range(3)] for mm in range(2)]

            def conv_tile(k, nm):
                g, j = k // 4, k % 4
                st = stpool.tile([KP, L], f16, tag="st", name=nm)
                if k % 4 == 1:
                    nc.scalar.activation(st[:], strip_sb[:, g, j * L:(j + 1) * L],
                                         AF.Copy)
                elif k % 4 == 3:
                    nc.gpsimd.tensor_copy(st[:], strip_sb[:, g, j * L:(j + 1) * L])
                else:
                    nc.vector.tensor_copy(st[:], strip_sb[:, g, j * L:(j + 1) * L])
                return st

            def gcn2_mm(k, lhs_tile, rw, first, last):
                st = conv_tile(k, f"st2_{k}")
                for mm in range(2):
                    lhsT = lhs_tile[:rw, mm * KP:(mm + 1) * KP]
                    for ci, (c0, c1) in enumerate(chg2):
                        nc.tensor.matmul(
                            ps2[mm][ci][:, :c1 - c0], lhsT,
                            st[:rw, c0:c1], start=first, stop=last)

            # own rows first (no gather dependency)
            for j in range(NLOC):
                rw = min(KP, ROWS - j * KP)
                gcn2_mm(MT + j, xw2l_sb[:, j, :], rw, j == 0, False)
            # gathered groups A, B, C as their collectives land
            # (rotating 8-deep buffer of gathered tiles)
            xg_cm = tc.tile_pool(name="xw2g", bufs=8)
            xgpool = xg_cm.__enter__()
            t0 = 0
            for gath, gt in ((gath_a, GA_T), (gath_b, GB_T), (gath_c, GC_T)):
                n = CORES * gt
                for t in range(n):
                    k = t0 + t
                    xt_g = xgpool.tile([KP, HID], f16, tag="xg", name=f"xg_{k}")
                    eng = nc.sync if t % 2 == 0 else nc.scalar
                    eng.dma_start(xt_g[:], gath[t * KP:(t + 1) * KP, :])
                    gcn2_mm(k, xt_g, KP, False, k == MT - 1)
                t0 += n
            xg_cm.__exit__(None, None, None)
            # h2 = relu(ps2 * dinv[d] + b2)
            h2t_sb = big.tile([KP, 2, L], f16)
            for mm in range(2):
                for ci, (c0, c1) in enumerate(chg2):
                    tt = tpool.tile([KP, 512], f16, tag="h2t")
                    nc.vector.tensor_tensor(tt[:, :c1 - c0],
                                            ps2[mm][ci][:, :c1 - c0],
                                            dinv2_sb[:, c0:c1], ALU.mult)
                    nc.scalar.activation(h2t_sb[:, mm, c0:c1],
                                         tt[:, :c1 - c0], AF.Relu,
                                         bias=b2c_sb[:, mm:mm + 1])

            st_cm.__exit__(None, None, None)
            sp_cm.__exit__(None, None, None)
            psG_cm.__exit__(None, None, None)
            gru_cm = tc.tile_pool(name="gru", bufs=1)
            gpool = gru_cm.__enter__()
            psg_cm = tc.tile_pool(name="ps", bufs=1, space="PSUM")
            pspool = psg_cm.__enter__()

            # ---- GI = W_ih @ h2T + (b_ih [+ b_hh for r,z]) ----
            ch512 = _chunks(L)
            gi_sb = gpool.tile([KP, 6, L], f16)
            for c0, c1 in ch512:
                psg = [pspool.tile([KP, 512], f32, tag=f"g{m}", name=f"psgi_{m}") for m in range(6)]
                for m in range(6):
                    for k in range(2):
                        nc.tensor.matmul(psg[m][:, :c1 - c0],
                                         wiht_sb[:, k, m * KP:(m + 1) * KP],
                                         h2t_sb[:, k, c0:c1],
                                         start=(k == 0), stop=(k == 1))
                    nc.scalar.activation(gi_sb[:, m, c0:c1], psg[m][:, :c1 - c0],
                                         AF.Identity, bias=gib_sb[:, m:m + 1])
            # per-core GI patch on the first HALO columns (core 0 kills its pads)
            for m in range(6):
                nc.vector.tensor_scalar(gi_sb[:, m, :HALO], gi_sb[:, m, :HALO],
                                        patch_sb[:, m:m + 1],
                                        patch_sb[:, 6 + m:7 + m],
                                        ALU.mult, ALU.add)

            # ---- GRU fixed-point sweeps ----
            # Double-buffered h (pure Jacobi): gates read hprev, chunked
            # scans (with carry) write hnew -- the scans overlap the next
            # chunk's matmuls on the PE with no aliasing stalls.
            hshA = gpool.tile([KP, 2, L + 1], f16, name="hshA")
            hshB = gpool.tile([KP, 2, L + 1], f16, name="hshB")
            for mm in range(2):
                nc.vector.memset(hshA[:, mm, :], 0.0)
                nc.vector.memset(hshB[:, mm, :], 0.0)
            for s in range(SWEEPS):
                hprev = hshA if s % 2 == 0 else hshB
                hnew = hshB if s % 2 == 0 else hshA
                z_sb = gpool.tile([KP, 2, L], f16, tag="Z")
                b_sb = gpool.tile([KP, 2, L], f16, tag="B")
                for ci, (c0, c1) in enumerate(ch512):
                    cw = c1 - c0
                    psg = [pspool.tile([KP, 512], f32, tag=f"g{m}",
                                       name=f"psu_{s}_{ci}_{m}")
                           for m in range(6)]
                    # u_rz = GI_rz (identity matmul) + W_hh_rz @ h_prev
                    for m in range(4):
                        nc.tensor.matmul(psg[m][:, :cw], ident_sb[:],
                                         gi_sb[:, m, c0:c1],
                                         start=True, stop=False)
                    for m in range(6):
                        for k in range(2):
                            nc.tensor.matmul(psg[m][:, :cw],
                                             whht_sb[:, k, m * KP:(m + 1) * KP],
                                             hprev[:, k, c0:c1],
                                             start=(m >= 4 and k == 0),
                                             stop=(k == 1))
                    for mm in range(2):
                        r_t = tpool.tile([KP, 512], f16, tag="r")
                        t_t = tpool.tile([KP, 512], f16, tag="t")
                        un_t = tpool.tile([KP, 512], f16, tag="un")
                        n_t = tpool.tile([KP, 512], f16, tag="n")
                        nc.scalar.activation(r_t[:, :cw], psg[mm][:, :cw],
                                             AF.Sigmoid)
                        nc.scalar.activation(z_sb[:, mm, c0:c1],
                                             psg[2 + mm][:, :cw], AF.Sigmoid)
                        # t = (gh_n + b_hh_n) * r  in one DVE op off PSUM
                        nc.vector.scalar_tensor_tensor(
                            t_t[:, :cw], psg[4 + mm][:, :cw],
                            bhn_sb[:, mm:mm + 1], r_t[:, :cw],
                            ALU.add, ALU.mult)
                        nc.vector.tensor_add(un_t[:, :cw], t_t[:, :cw],
                                             gi_sb[:, 4 + mm, c0:c1])
                        nc.scalar.activation(n_t[:, :cw], un_t[:, :cw], AF.Tanh)
                        # b2 = (z-1)*n; the scan uses op1=subtract so
                        # h = z*h_prev - b2 = z*h_prev + (1-z)*n
                        nc.vector.scalar_tensor_tensor(
                            b_sb[:, mm, c0:c1], z_sb[:, mm, c0:c1], 1.0,
                            n_t[:, :cw], ALU.subtract, ALU.mult)
                    # chunk scans with carry: h_t = z_t*h_{t-1} + (1-z_t)n_t
                    for mm in range(2):
                        init = 0.0 if ci == 0 else hnew[:, mm, c0:c0 + 1]
                        nc.vector.tensor_tensor_scan(
                            hnew[:, mm, c0 + 1:c1 + 1], z_sb[:, mm, c0:c1],
                            b_sb[:, mm, c0:c1], init, ALU.mult, ALU.subtract)
            hsh_sb = hshA if SWEEPS % 2 == 0 else hshB

            psg_cm.__exit__(None, None, None)
            gru_cm.__exit__(None, None, None)

            # ---- final Linear on the real rows (skip halo) ----
            out_sb = cpool.tile([4, ROWS], f32)
            for c0, c1 in _chunks(ROWS):
                cw = c1 - c0
                psf = psxw.tile([KP, 512], f32, tag="xwps")
                for k in range(2):
                    nc.tensor.matmul(psf[:OUT, :cw], fcwt_sb[:, k, :],
                                     hsh_sb[:, k, HALO + 1 + c0:HALO + 1 + c1],
                                     start=(k == 0), stop=(k == 1))
                nc.scalar.activation(out_sb[:OUT, c0:c1], psf[:OUT, :cw],
                                     AF.Identity, bias=fcb_sb[:OUT, :])
            nc.sync.dma_start(out_d[:], out_sb[:OUT, :])

    nc.compile()
    return nc


def host_prepare(inputs):
    """Build the per-core input maps from the full problem inputs."""
    import ml_dtypes

    x = np.asarray(inputs["x"], np.float32)
    ei = np.asarray(inputs["edge_index"])
    W1 = np.asarray(inputs["W1"], np.float32)
    b1 = np.asarray(inputs["b1"], np.float32)
    W2 = np.asarray(inputs["W2"], np.float32)
    b2 = np.asarray(inputs["b2"], np.float32)
    W_ih = np.asarray(inputs["W_ih"], np.float32)
    W_hh = np.asarray(inputs["W_hh"], np.float32)
    b_ih = np.asarray(inputs["b_ih"], np.float32)
    b_hh = np.asarray(inputs["b_hh"], np.float32)
    fc_w = np.asarray(inputs["fc_w"], np.float32)
    fc_b = np.asarray(inputs["fc_b"], np.float32)

    N = NUM_NODES
    src, dst = ei[0].astype(np.int64), ei[1].astype(np.int64)
    deg = np.bincount(dst, minlength=N).astype(np.float64) + 1.0
    dinv = (1.0 / np.sqrt(deg)).astype(np.float32)
    # integer multiplicity matrix M[s, d] (+1 on the diagonal, self loops)
    Mi = np.zeros((N, N), np.float32)
    np.add.at(Mi, (src, dst), 1.0)
    idx = np.arange(N)
    Mi[idx, idx] += 1.0

    # strip row order: grpA (2 tiles/core), grpB (4), grpC (4, last padded),
    # then the append block (own rows).  Same order for xd.
    perm = np.full(MT * KP, -1, np.int64)  # global-block strip row -> node
    t = 0
    for c in range(CORES):           # grpA
        base = c * ROWS
        for i in range(GA_T):
            perm[t * KP:(t + 1) * KP] = base + i * KP + np.arange(KP)
            t += 1
    for c in range(CORES):           # grpB
        base = c * ROWS + GA_R
        for i in range(GB_T):
            perm[t * KP:(t + 1) * KP] = base + i * KP + np.arange(KP)
            t += 1
    for c in range(CORES):           # grpC (482 real rows -> 4 padded tiles)
        base = c * ROWS + GA_R + GB_R
        for i in range(GC_T):
            rw = min(KP, GC_R - i * KP)
            perm[t * KP:t * KP + rw] = base + i * KP + np.arange(rw)
            t += 1
    assert t == MT

    xdf = (x * dinv[:, None]).astype(np.float16)   # [N, 64]

    common = {
        "w1": W1.astype(np.float16),
        "w2": W2.astype(np.float16),
        "wiht": W_ih.T.astype(np.float16),
        "whht": W_hh.T.astype(np.float16),
        "fcwt": fc_w.T.astype(np.float16),
        "ident": np.eye(KP, dtype=np.float16),
        "b1c": b1.reshape(2, KP).T.astype(np.float32).copy(),
        "b2c": b2.reshape(2, KP).T.astype(np.float32).copy(),
        "gib": (b_ih + np.concatenate([b_hh[:2 * HID],
                                       np.zeros(HID, np.float32)])
                ).reshape(6, KP).T.astype(np.float32).copy(),
        "bhn": b_hh[2 * HID:].reshape(2, KP).T.astype(np.float32).copy(),
        "fcb": np.concatenate([fc_b, np.zeros(KP - OUT, np.float32)]
                              ).reshape(KP, 1),
    }

    in_maps = []
    for c in range(CORES):
        r0, r1 = c * ROWS, (c + 1) * ROWS
        # column window [r0-HALO, r1); core 0's first 64 cols are zero pads
        a2t = np.zeros((GRP * 4 * KP, L), np.float32)
        if c == 0:
            colw = np.zeros((N, L), np.float32)
            colw[:, HALO:] = Mi[:, r0:r1]
        else:
            colw = Mi[:, r0 - HALO:r1]
        # global block in permuted row order, own rows zeroed
        gl = colw[np.maximum(perm, 0)]       # [MT*KP, L]
        own = (perm >= r0) & (perm < r1)
        gl[own] = 0.0
        gl[perm < 0] = 0.0
        a2t[:MT * KP] = gl
        # append block: own rows
        a2t[MT * KP:MT * KP + ROWS] = colw[r0:r1]
        # 4-way K-tile interleave: row g*128+p, col j*L+cc <- tile (4g+j) row p
        a2t = np.ascontiguousarray(
            a2t.reshape(GRP, 4, KP, L).transpose(0, 2, 1, 3)
        ).reshape(GRP * KP, 4 * L).astype(ml_dtypes.float8_e4m3)

        # xd in the same strip row order: [128 part, MT2, 64]
        xda = np.zeros((MT2 * KP, IN_FEAT), np.float16)
        xda[:MT * KP][perm >= 0] = xdf[perm[perm >= 0]]
        xda[MT * KP:MT * KP + ROWS] = xdf[r0:r1]
        xd = np.ascontiguousarray(
            xda.reshape(MT2, KP, IN_FEAT).transpose(1, 0, 2)
        ).reshape(KP, MT2 * IN_FEAT)

        dinv1 = np.broadcast_to(dinv[r0:r1], (IN_FEAT, ROWS)
                                ).astype(np.float16).copy()
        dv2 = np.zeros(L, np.float32)
        if c == 0:
            dv2[HALO:] = dinv[r0:r1]
        else:
            dv2[:] = dinv[r0 - HALO:r1]
        dinv2 = np.broadcast_to(dv2, (KP, L)).astype(np.float16).copy()
        dloc = np.zeros((NLOC, KP), np.float32)
        dloc.reshape(-1)[:ROWS] = dinv[r0:r1]
        dinvl = dloc.T.copy()                # [128, NLOC]

        patch = np.zeros((KP, 12), np.float32)
        if c == 0:
            # mul=0; add=-60 for r,z gate tiles, 0 for n tiles -> pad cols
            # produce exactly h=0 so row 0 starts from the true h0=0.
            patch[:, 6:10] = -60.0
        else:
            patch[:, 0:6] = 1.0
        in_maps.append({**common, "a2t": a2t, "xd": xd, "dinv1": dinv1,
                        "dinv2": dinv2, "dinvl": dinvl, "patch": patch})
    return in_maps


def assemble_output(results):
    outs = [r["out_t"].T for r in results]          # each [ROWS, OUT]
    full = np.concatenate(outs, axis=0).astype(np.float32)
    return full[None]                               # [1, N, OUT]


def kernel(**inputs) -> np.ndarray:
    from concourse import bass_utils

    if "nc" not in _CACHE:
        _CACHE["nc"] = build_program()
    nc = _CACHE["nc"]
    in_maps = host_prepare(inputs)
    res = bass_utils.run_bass_kernel_spmd(
        nc, in_maps, core_ids=list(range(CORES)))
    return assemble_output(res.results)


if __name__ == "__main__":
    import reference

    inputs = {k: np.asarray(v) for k, v in reference.setup_inputs().items()}
    out = kernel(**inputs)
    print("kernel out", out.shape, out.dtype)
    np.save("/root/problem/kernel_out.npy", out)


# revision 14
# speedup vs baseline: 1.2164x; 1.2164x over previous
"""DCRNN (2x GCNConv + GRU-over-nodes + Linear) on 8 Trainium2 cores.

Strategy (v2)
-------------
* The normalized adjacency A = D^-1/2 (Adj+I) D^-1/2 is factored as
  dinv[s] * M[s,d] * dinv[d] with M a small-integer matrix (edge
  multiplicity + self loops).  M is shipped as an fp8e4 strip (integers
  are exact in fp8), loaded ONCE into SBUF (15MB resident) and expanded
  to fp16 tile-by-tile on the otherwise-idle Vector/Scalar engines.
  dinv[s] is folded into the dense operand (x resp. XW2); dinv[d] is a
  per-column multiply fused into the PSUM evacuation.
* GCN1 uses associativity: A @ (x @ W1) = (A @ x) @ W1 -- the aggregate
  runs over the 64-wide x (4x less PE work than aggregating 256-wide
  XW1), then a tiny K=64 matmul applies W1.
* GCN2: each core computes h2 for its 1250 rows + 64-row left halo from
  the gathered XW2.  The XW2 AllGather is split into 3 src-group
  collectives (2/4/4 tiles per core) so the GCN2 K-loop can start on
  group A while groups B/C are still in flight; the core's own rows are
  processed first (no gather dependency at all).
* GRU over the 10000-node sequence: K Jacobi sweeps; gates from the
  previous sweep's h, then the exact affine scan (tensor_tensor_scan).
  The 64-row halo makes cores independent (contraction ~0.74/step).
* Final Linear on the node shard; host concatenates the 8 shards.
"""

import numpy as np

NUM_NODES = 10000
IN_FEAT = 64
HID = 256
OUT = 3
CORES = 8
ROWS = NUM_NODES // CORES          # 1250
HALO = 64
L = ROWS + HALO                    # 1314 local sequence length
SWEEPS = 8
KP = 128
# global strip tiles: grpA 16 (2/core) + grpB 32 (4/core) + grpC 32 (4/core,
# last tile of each core zero-padded 98->128) = 80; append (own rows) 10
MT = 80
NLOC = 10
MT2 = MT + NLOC                    # 90 K-tiles
GRP = (MT2 + 3) // 4               # 23 interleave groups of 4 K-tiles
# src-group split (in 128-row tiles per core)
GA_T, GB_T, GC_T = 2, 4, 4         # tiles per core in groups A, B, C
GA_R, GB_R = 256, 512              # real rows per core in A, B
GC_R = ROWS - GA_R - GB_R          # 482 real rows in C (padded to 512)

_CACHE = {}


def _chunks(total, step=512):
    return [(c, min(c + step, total)) for c in range(0, total, step)]


def build_program():
    import concourse.bass as bass
    import concourse.mybir as mybir
    import concourse.tile as tile
    from concourse import bacc

    f16 = mybir.dt.float16
    f8 = mybir.dt.float8e4
    f32 = mybir.dt.float32
    AF = mybir.ActivationFunctionType
    ALU = mybir.AluOpType

    nc = bacc.Bacc("TRN2", num_devices=CORES)

    # ---- inputs ----
    # fp8 integer strip, 4-way K-tile interleaved: row (g*128+p), col
    # (j*L+c) holds M[src-of-tile(4g+j) p, window col c].
    a2t_d = nc.dram_tensor("a2t", [GRP * KP, 4 * L], f8, kind="ExternalInput")
    # xd = x * dinv[s], strip row order, [128 part, MT2*64]
    xd_d = nc.dram_tensor("xd", [KP, MT2 * IN_FEAT], f16, kind="ExternalInput")
    w1_d = nc.dram_tensor("w1", [IN_FEAT, HID], f16, kind="ExternalInput")
    w2_d = nc.dram_tensor("w2", [HID, HID], f16, kind="ExternalInput")
    wiht_d = nc.dram_tensor("wiht", [HID, 3 * HID], f16, kind="ExternalInput")
    whht_d = nc.dram_tensor("whht", [HID, 3 * HID], f16, kind="ExternalInput")
    fcwt_d = nc.dram_tensor("fcwt", [HID, OUT], f16, kind="ExternalInput")
    ident_d = nc.dram_tensor("ident", [KP, KP], f16, kind="ExternalInput")
    b1c_d = nc.dram_tensor("b1c", [KP, 2], f32, kind="ExternalInput")
    b2c_d = nc.dram_tensor("b2c", [KP, 2], f32, kind="ExternalInput")
    gib_d = nc.dram_tensor("gib", [KP, 6], f32, kind="ExternalInput")
    bhn_d = nc.dram_tensor("bhn", [KP, 2], f32, kind="ExternalInput")
    fcb_d = nc.dram_tensor("fcb", [KP, 1], f32, kind="ExternalInput")
    patch_d = nc.dram_tensor("patch", [KP, 12], f32, kind="ExternalInput")
    dinv1_d = nc.dram_tensor("dinv1", [IN_FEAT, ROWS], f16, kind="ExternalInput")
    dinv2_d = nc.dram_tensor("dinv2", [KP, L], f16, kind="ExternalInput")
    dinvl_d = nc.dram_tensor("dinvl", [KP, NLOC], f32, kind="ExternalInput")
    out_d = nc.dram_tensor("out_t", [OUT, ROWS], f32, kind="ExternalOutput")

    with tile.TileContext(nc) as tc:
        with (
            tc.tile_pool(name="const", bufs=1) as cpool,
            tc.tile_pool(name="big", bufs=1) as big,
            tc.tile_pool(name="tmp", bufs=3) as tpool,
            tc.tile_pool(name="psxw", bufs=2, space="PSUM") as psxw,
            tc.tile_pool(name="dram", bufs=1, space="DRAM") as dpool,
        ):
            sp_cm = tc.tile_pool(name="strip", bufs=1)
            spool = sp_cm.__enter__()
            st_cm = tc.tile_pool(name="stage", bufs=3)
            stpool = st_cm.__enter__()
            # ---- load constants ----
            w1_sb = cpool.tile([IN_FEAT, HID], f16)
            w2_sb = cpool.tile([KP, 2, HID], f16)
            wiht_sb = cpool.tile([KP, 2, 3 * HID], f16)
            whht_sb = cpool.tile([KP, 2, 3 * HID], f16)
            fcwt_sb = cpool.tile([KP, 2, OUT], f16)
            ident_sb = cpool.tile([KP, KP], f16)
            b1c_sb = cpool.tile([KP, 2], f32)
            b2c_sb = cpool.tile([KP, 2], f32)
            gib_sb = cpool.tile([KP, 6], f32)
            bhn_sb = cpool.tile([KP, 2], f32)
            fcb_sb = cpool.tile([KP, 1], f32)
            patch_sb = cpool.tile([KP, 12], f32)
            dinv1_sb = cpool.tile([IN_FEAT, ROWS], f16)
            dinv2_sb = cpool.tile([KP, L], f16)
            dinvl_sb = cpool.tile([KP, NLOC], f32)

            nc.sync.dma_start(ident_sb[:], ident_d[:])
            nc.sync.dma_start(w1_sb[:], w1_d[:])
            for k in range(2):
                nc.sync.dma_start(w2_sb[:, k, :], w2_d[k * KP:(k + 1) * KP, :])
                nc.sync.dma_start(wiht_sb[:, k, :], wiht_d[k * KP:(k + 1) * KP, :])
                nc.sync.dma_start(whht_sb[:, k, :], whht_d[k * KP:(k + 1) * KP, :])
                nc.sync.dma_start(fcwt_sb[:, k, :], fcwt_d[k * KP:(k + 1) * KP, :])
            nc.sync.dma_start(b1c_sb[:], b1c_d[:])
            nc.sync.dma_start(b2c_sb[:], b2c_d[:])
            nc.sync.dma_start(gib_sb[:], gib_d[:])
            nc.sync.dma_start(bhn_sb[:], bhn_d[:])
            nc.sync.dma_start(fcb_sb[:], fcb_d[:])
            nc.sync.dma_start(patch_sb[:], patch_d[:])
            nc.sync.dma_start(dinv1_sb[:], dinv1_d[:])
            nc.sync.dma_start(dinv2_sb[:], dinv2_d[:])
            nc.sync.dma_start(dinvl_sb[:], dinvl_d[:])

            # ---- xd first (GCN1's stationary operand), then the fp8 strip ----
            xd_cm = tc.tile_pool(name="xdp", bufs=1)
            xdp = xd_cm.__enter__()
            xd_sb = xdp.tile([KP, MT2, IN_FEAT], f16)
            nc.scalar.dma_start(xd_sb[:], xd_d[:])
            strip_sb = spool.tile([KP, GRP, 4 * L], f8)
            for g in range(GRP):
                eng = nc.sync if g % 2 == 0 else nc.scalar
                eng.dma_start(strip_sb[:, g, :], a2t_d[g * KP:(g + 1) * KP, :])

            psG_cm = tc.tile_pool(name="psG", bufs=1, space="PSUM")
            psG = psG_cm.__enter__()

            # tiny AllGather to absorb the first-collective ncfw setup cost
            ccw_in = dpool.tile([CORES, 64], f16)
            ccw_out = dpool.tile([CORES * CORES, 64], f16, addr_space="Shared")
            nc.sync.dma_start(ccw_in[0:KP // 16, :], ident_sb[0:8, 0:64])
            nc.gpsimd.collective_compute(
                "AllGather", mybir.AluOpType.bypass,
                replica_groups=[list(range(CORES))],
                ins=[ccw_in.opt()], outs=[ccw_out.opt()])

            # PE warm-up burst so the HAM clock-gate opens before GCN1
            for i in range(16):
                psd = psxw.tile([KP, 512], f32, tag="xwps", name=f"warm_{i}")
                nc.tensor.matmul(psd[:, :KP], ident_sb[:], ident_sb[:],
                                 start=True, stop=True)

            # ---- GCN1: agg1 = (M^T @ xd) on own cols, then h1 = relu(...) ----
            chg1 = _chunks(ROWS)
            ps1 = [psG.tile([KP, 512], f32, tag=f"G{ci}", name=f"ps1_{ci}")
                   for ci in range(3)]
            for k in range(MT2):
                g, j = k // 4, k % 4
                st = stpool.tile([KP, L], f16, tag="st")
                if k % 5 in (1, 3):
                    nc.scalar.activation(st[:, :ROWS],
                                         strip_sb[:, g, j * L + HALO:(j + 1) * L],
                                         AF.Copy)
                else:
                    nc.vector.tensor_copy(st[:, :ROWS],
                                          strip_sb[:, g, j * L + HALO:(j + 1) * L])
                for ci, (c0, c1) in enumerate(chg1):
                    nc.tensor.matmul(ps1[ci][:IN_FEAT, :c1 - c0],
                                     xd_sb[:, k, :], st[:, c0:c1],
                                     start=(k == 0), stop=(k == MT2 - 1))
            # agg1_sb = ps1 * dinv[d] (column scale)
            agg1_sb = big.tile([IN_FEAT, ROWS], f16)
            for ci, (c0, c1) in enumerate(chg1):
                nc.vector.tensor_tensor(agg1_sb[:, c0:c1],
                                        ps1[ci][:IN_FEAT, :c1 - c0],
                                        dinv1_sb[:, c0:c1], ALU.mult)
            # h1T = relu(W1h^T @ agg1 + b1)
            h1t_sb = big.tile([KP, 2, ROWS], f16)
            for mm in range(2):
                for ci, (c0, c1) in enumerate(chg1):
                    psw = psxw.tile([KP, 512], f32, tag="xwps")
                    nc.tensor.matmul(psw[:, :c1 - c0],
                                     w1_sb[:, mm * KP:(mm + 1) * KP],
                                     agg1_sb[:, c0:c1], start=True, stop=True)
                    nc.scalar.activation(h1t_sb[:, mm, c0:c1],
                                         psw[:, :c1 - c0], AF.Relu,
                                         bias=b1c_sb[:, mm:mm + 1])
            xd_cm.__exit__(None, None, None)

            # ---- XW2 shard = (h1 @ W2) * dinv[s]  (natural layout) ----
            xw2l_sb = big.tile([KP, NLOC, HID], f16)
            for j in range(NLOC):
                rw = min(KP, ROWS - j * KP)
                ps = psxw.tile([KP, 512], f32, tag="xwps")
                for k in range(2):
                    nc.tensor.matmul(ps[:rw, :HID],
                                     h1t_sb[:, k, j * KP:j * KP + rw],
                                     w2_sb[:, k, :],
                                     start=(k == 0), stop=(k == 1))
                nc.scalar.activation(xw2l_sb[:rw, j, :], ps[:rw, :HID],
                                     AF.Copy, scale=dinvl_sb[:rw, j:j + 1])

            # ---- three src-group AllGathers of XW2D ----
            zer_sb = cpool.tile([32, HID], f16)
            nc.vector.memset(zer_sb[:], 0.0)
            bounce_a = dpool.tile([GA_T * KP, HID], f16)
            bounce_b = dpool.tile([GB_T * KP, HID], f16)
            bounce_c = dpool.tile([GC_T * KP, HID], f16)
            gath_a = dpool.tile([CORES * GA_T * KP, HID], f16, addr_space="Shared")
            gath_b = dpool.tile([CORES * GB_T * KP, HID], f16, addr_space="Shared")
            gath_c = dpool.tile([CORES * GC_T * KP, HID], f16, addr_space="Shared")
            for j in range(GA_T):
                nc.sync.dma_start(bounce_a[j * KP:(j + 1) * KP, :],
                                  xw2l_sb[:, j, :])
            for j in range(GB_T):
                nc.sync.dma_start(bounce_b[j * KP:(j + 1) * KP, :],
                                  xw2l_sb[:, GA_T + j, :])
            for j in range(GC_T):
                jj = GA_T + GB_T + j
                rw = min(KP, ROWS - jj * KP)
                nc.sync.dma_start(bounce_c[j * KP:j * KP + rw, :],
                                  xw2l_sb[:rw, jj, :])
                if rw < KP:
                    nc.sync.dma_start(bounce_c[j * KP + rw:(j + 1) * KP, :],
                                      zer_sb[:KP - rw, :])
            for bb, gg in ((bounce_a, gath_a), (bounce_b, gath_b),
                           (bounce_c, gath_c)):
                nc.gpsimd.collective_compute(
                    "AllGather", mybir.AluOpType.bypass,
                    replica_groups=[list(range(CORES))],
                    ins=[bb.opt()], outs=[gg.opt()])

            # ---- GCN2 over the extended (halo) shard ----
            chg2 = _chunks(L)
            ps2 = [[psG.tile([KP, 512], f32,
                             tag=f"G{mm * 3 + ci}", name=f"ps2_{mm}_{ci}")
                    for ci in range(3)] for mm in range(2)]

            def conv_tile(k, nm):
                g, j = k // 4, k % 4
                st = stpool.tile([KP, L], f16, tag="st", name=nm)
                if k % 5 in (1, 3):
                    nc.scalar.activation(st[:], strip_sb[:, g, j * L:(j + 1) * L],
                                         AF.Copy)
                else:
                    nc.vector.tensor_copy(st[:], strip_sb[:, g, j * L:(j + 1) * L])
                return st

            def gcn2_mm(k, lhs_tile, rw, first, last):
                st = conv_tile(k, f"st2_{k}")
                for mm in range(2):
                    lhsT = lhs_tile[:rw, mm * KP:(mm + 1) * KP]
                    for ci, (c0, c1) in enumerate(chg2):
                        nc.tensor.matmul(
                            ps2[mm][ci][:, :c1 - c0], lhsT,
                            st[:rw, c0:c1], start=first, stop=last)

            # own rows first (no gather dependency)
            for j in range(NLOC):
                rw = min(KP, ROWS - j * KP)
                gcn2_mm(MT + j, xw2l_sb[:, j, :], rw, j == 0, False)
            # gathered groups A, B, C as their collectives land
            # (rotating 8-deep buffer of gathered tiles)
            xg_cm = tc.tile_pool(name="xw2g", bufs=8)
            xgpool = xg_cm.__enter__()
            t0 = 0
            for gath, gt in ((gath_a, GA_T), (gath_b, GB_T), (gath_c, GC_T)):
                n = CORES * gt
                for t in range(n):
                    k = t0 + t
                    xt_g = xgpool.tile([KP, HID], f16, tag="xg", name=f"xg_{k}")
                    eng = nc.sync if t % 2 == 0 else nc.scalar
                    eng.dma_start(xt_g[:], gath[t * KP:(t + 1) * KP, :])
                    gcn2_mm(k, xt_g, KP, False, k == MT - 1)
                t0 += n
            xg_cm.__exit__(None, None, None)
            # h2 = relu(ps2 * dinv[d] + b2)
            h2t_sb = big.tile([KP, 2, L], f16)
            for mm in range(2):
                for ci, (c0, c1) in enumerate(chg2):
                    tt = tpool.tile([KP, 512], f16, tag="h2t")
                    nc.vector.tensor_tensor(tt[:, :c1 - c0],
                                            ps2[mm][ci][:, :c1 - c0],
                                            dinv2_sb[:, c0:c1], ALU.mult)
                    nc.scalar.activation(h2t_sb[:, mm, c0:c1],
                                         tt[:, :c1 - c0], AF.Relu,
                                         bias=b2c_sb[:, mm:mm + 1])

            st_cm.__exit__(None, None, None)
            sp_cm.__exit__(None, None, None)
            psG_cm.__exit__(None, None, None)
            gru_cm = tc.tile_pool(name="gru", bufs=1)
            gpool = gru_cm.__enter__()
            psg_cm = tc.tile_pool(name="ps", bufs=1, space="PSUM")
            pspool = psg_cm.__enter__()

            # ---- GI = W_ih @ h2T + (b_ih [+ b_hh for r,z]) ----
            ch512 = _chunks(L)
            gi_sb = gpool.tile([KP, 6, L], f16)
            for c0, c1 in ch512:
                psg = [pspool.tile([KP, 512], f32, tag=f"g{m}", name=f"psgi_{m}") for m in range(6)]
                for m in range(6):
                    for k in range(2):
                        nc.tensor.matmul(psg[m][:, :c1 - c0],
                                         wiht_sb[:, k, m * KP:(m + 1) * KP],
                                         h2t_sb[:, k, c0:c1],
                                         start=(k == 0), stop=(k == 1))
                    nc.scalar.activation(gi_sb[:, m, c0:c1], psg[m][:, :c1 - c0],
                                         AF.Identity, bias=gib_sb[:, m:m + 1])
            # per-core GI patch on the first HALO columns (core 0 kills its pads)
            for m in range(6):
                nc.vector.tensor_scalar(gi_sb[:, m, :HALO], gi_sb[:, m, :HALO],
                                        patch_sb[:, m:m + 1],
                                        patch_sb[:, 6 + m:7 + m],
                                        ALU.mult, ALU.add)

            # ---- GRU fixed-point sweeps ----
            # Double-buffered h (pure Jacobi): gates read hprev, chunked
            # scans (with carry) write hnew -- the scans overlap the next
            # chunk's matmuls on the PE with no aliasing stalls.
            hshA = gpool.tile([KP, 2, L + 1], f16, name="hshA")
            hshB = gpool.tile([KP, 2, L + 1], f16, name="hshB")
            for mm in range(2):
                nc.vector.memset(hshA[:, mm, :], 0.0)
                nc.vector.memset(hshB[:, mm, :], 0.0)
            for s in range(SWEEPS):
                hprev = hshA if s % 2 == 0 else hshB
                hnew = hshB if s % 2 == 0 else hshA
                z_sb = gpool.tile([KP, 2, L], f16, tag="Z")
                b_sb = gpool.tile([KP, 2, L], f16, tag="B")
                for ci, (c0, c1) in enumerate(ch512):
                    cw = c1 - c0
                    psg = [pspool.tile([KP, 512], f32, tag=f"g{m}",
                                       name=f"psu_{s}_{ci}_{m}")
                           for m in range(6)]
                    # u_rz = GI_rz (identity matmul) + W_hh_rz @ h_prev
                    for m in range(4):
                        nc.tensor.matmul(psg[m][:, :cw], ident_sb[:],
                                         gi_sb[:, m, c0:c1],
                                         start=True, stop=False)
                    for m in range(6):
                        for k in range(2):
                            nc.tensor.matmul(psg[m][:, :cw],
                                             whht_sb[:, k, m * KP:(m + 1) * KP],
                                             hprev[:, k, c0:c1],
                                             start=(m >= 4 and k == 0),
                                             stop=(k == 1))
                    for mm in range(2):
                        r_t = tpool.tile([KP, 512], f16, tag="r")
                        t_t = tpool.tile([KP, 512], f16, tag="t")
                        un_t = tpool.tile([KP, 512], f16, tag="un")
                        n_t = tpool.tile([KP, 512], f16, tag="n")
                        nc.scalar.activation(r_t[:, :cw], psg[mm][:, :cw],
                                             AF.Sigmoid)
                        nc.scalar.activation(z_sb[:, mm, c0:c1],
                                             psg[2 + mm][:, :cw], AF.Sigmoid)
                        # t = (gh_n + b_hh_n) * r  in one DVE op off PSUM
                        nc.vector.scalar_tensor_tensor(
                            t_t[:, :cw], psg[4 + mm][:, :cw],
                            bhn_sb[:, mm:mm + 1], r_t[:, :cw],
                            ALU.add, ALU.mult)
                        eng_un = nc.vector if (ci == 2 and mm == 1) else nc.gpsimd
                        eng_un.tensor_add(un_t[:, :cw], t_t[:, :cw],
                                          gi_sb[:, 4 + mm, c0:c1])
                        nc.scalar.activation(n_t[:, :cw], un_t[:, :cw], AF.Tanh)
                        # b2 = (z-1)*n; the scan uses op1=subtract so
                        # h = z*h_prev - b2 = z*h_prev + (1-z)*n
                        nc.vector.scalar_tensor_tensor(
                            b_sb[:, mm, c0:c1], z_sb[:, mm, c0:c1], 1.0,
                            n_t[:, :cw], ALU.subtract, ALU.mult)
                    # chunk scans with carry: h_t = z_t*h_{t-1} + (1-z_t)n_t
                    for mm in range(2):
                        init = 0.0 if ci == 0 else hnew[:, mm, c0:c0 + 1]
                        nc.vector.tensor_tensor_scan(
                            hnew[:, mm, c0 + 1:c1 + 1], z_sb[:, mm, c0:c1],
                            b_sb[:, mm, c0:c1], init, ALU.mult, ALU.subtract)
            hsh_sb = hshA if SWEEPS % 2 == 0 else hshB

            psg_cm.__exit__(None, None, None)
            gru_cm.__exit__(None, None, None)

            # ---- final Linear on the real rows (skip halo) ----
            out_sb = cpool.tile([4, ROWS], f32)
            for c0, c1 in _chunks(ROWS):
                cw = c1 - c0
                psf = psxw.tile([KP, 512], f32, tag="xwps")
                for k in range(2):
                    nc.tensor.matmul(psf[:OUT, :cw], fcwt_sb[:, k, :],
                                     hsh_sb[:, k, HALO + 1 + c0:HALO + 1 + c1],
                                     start=(k == 0), stop=(k == 1))
                nc.scalar.activation(out_sb[:OUT, c0:c1], psf[:OUT, :cw],
                                     AF.Identity, bias=fcb_sb[:OUT, :])
            nc.sync.dma_start(out_d[:], out_sb[:OUT, :])

    nc.compile()
    return nc


def host_prepare(inputs):
    """Build the per-core input maps from the full problem inputs."""
    import ml_dtypes

    x = np.asarray(inputs["x"], np.float32)
    ei = np.asarray(inputs["edge_index"])
    W1 = np.asarray(inputs["W1"], np.float32)
    b1 = np.asarray(inputs["b1"], np.float32)
    W2 = np.asarray(inputs["W2"], np.float32)
    b2 = np.asarray(inputs["b2"], np.float32)
    W_ih = np.asarray(inputs["W_ih"], np.float32)
    W_hh = np.asarray(inputs["W_hh"], np.float32)
    b_ih = np.asarray(inputs["b_ih"], np.float32)
    b_hh = np.asarray(inputs["b_hh"], np.float32)
    fc_w = np.asarray(inputs["fc_w"], np.float32)
    fc_b = np.asarray(inputs["fc_b"], np.float32)

    N = NUM_NODES
    src, dst = ei[0].astype(np.int64), ei[1].astype(np.int64)
    deg = np.bincount(dst, minlength=N).astype(np.float64) + 1.0
    dinv = (1.0 / np.sqrt(deg)).astype(np.float32)
    # integer multiplicity matrix M[s, d] (+1 on the diagonal, self loops)
    Mi = np.zeros((N, N), np.float32)
    np.add.at(Mi, (src, dst), 1.0)
    idx = np.arange(N)
    Mi[idx, idx] += 1.0

    # strip row order: grpA (2 tiles/core), grpB (4), grpC (4, last padded),
    # then the append block (own rows).  Same order for xd.
    perm = np.full(MT * KP, -1, np.int64)  # global-block strip row -> node
    t = 0
    for c in range(CORES):           # grpA
        base = c * ROWS
        for i in range(GA_T):
            perm[t * KP:(t + 1) * KP] = base + i * KP + np.arange(KP)
            t += 1
    for c in range(CORES):           # grpB
        base = c * ROWS + GA_R
        for i in range(GB_T):
            perm[t * KP:(t + 1) * KP] = base + i * KP + np.arange(KP)
            t += 1
    for c in range(CORES):           # grpC (482 real rows -> 4 padded tiles)
        base = c * ROWS + GA_R + GB_R
        for i in range(GC_T):
            rw = min(KP, GC_R - i * KP)
            perm[t * KP:t * KP + rw] = base + i * KP + np.arange(rw)
            t += 1
    assert t == MT

    xdf = (x * dinv[:, None]).astype(np.float16)   # [N, 64]

    common = {
        "w1": W1.astype(np.float16),
        "w2": W2.astype(np.float16),
        "wiht": W_ih.T.astype(np.float16),
        "whht": W_hh.T.astype(np.float16),
        "fcwt": fc_w.T.astype(np.float16),
        "ident": np.eye(KP, dtype=np.float16),
        "b1c": b1.reshape(2, KP).T.astype(np.float32).copy(),
        "b2c": b2.reshape(2, KP).T.astype(np.float32).copy(),
        "gib": (b_ih + np.concatenate([b_hh[:2 * HID],
                                       np.zeros(HID, np.float32)])
                ).reshape(6, KP).T.astype(np.float32).copy(),
        "bhn": b_hh[2 * HID:].reshape(2, KP).T.astype(np.float32).copy(),
        "fcb": np.concatenate([fc_b, np.zeros(KP - OUT, np.float32)]
                              ).reshape(KP, 1),
    }

    in_maps = []
    for c in range(CORES):
        r0, r1 = c * ROWS, (c + 1) * ROWS
        # column window [r0-HALO, r1); core 0's first 64 cols are zero pads
        a2t = np.zeros((GRP * 4 * KP, L), np.float32)
        if c == 0:
            colw = np.zeros((N, L), np.float32)
            colw[:, HALO:] = Mi[:, r0:r1]
        else:
            colw = Mi[:, r0 - HALO:r1]
        # global block in permuted row order, own rows zeroed
        gl = colw[np.maximum(perm, 0)]       # [MT*KP, L]
        own = (perm >= r0) & (perm < r1)
        gl[own] = 0.0
        gl[perm < 0] = 0.0
        a2t[:MT * KP] = gl
        # append block: own rows
        a2t[MT * KP:MT * KP + ROWS] = colw[r0:r1]
        # 4-way K-tile interleave: row g*128+p, col j*L+cc <- tile (4g+j) row p
        a2t = np.ascontiguousarray(
            a2t.reshape(GRP, 4, KP, L).transpose(0, 2, 1, 3)
        ).reshape(GRP * KP, 4 * L).astype(ml_dtypes.float8_e4m3)

        # xd in the same strip row order: [128 part, MT2, 64]
        xda = np.zeros((MT2 * KP, IN_FEAT), np.float16)
        xda[:MT * KP][perm >= 0] = xdf[perm[perm >= 0]]
        xda[MT * KP:MT * KP + ROWS] = xdf[r0:r1]
        xd = np.ascontiguousarray(
            xda.reshape(MT2, KP, IN_FEAT).transpose(1, 0, 2)
        ).reshape(KP, MT2 * IN_FEAT)

        dinv1 = np.broadcast_to(dinv[r0:r1], (IN_FEAT, ROWS)
                                ).astype(np.float16).copy()
        dv2 = np.zeros(L, np.float32)
        if c == 0:
            dv2[HALO:] = dinv[r0:r1]
        else:
            dv2[:] = dinv[r0 - HALO:r1]
        dinv2 = np.broadcast_to(dv2, (KP, L)).astype(np.float16).copy()
        dloc = np.zeros((NLOC, KP), np.float32)
        dloc.reshape(-1)[:ROWS] = dinv[r0:r1]
        dinvl = dloc.T.copy()                # [128, NLOC]

        patch = np.zeros((KP, 12), np.float32)
        if c == 0:
            # mul=0; add=-60 for r,z gate tiles, 0 for n tiles -> pad cols
            # produce exactly h=0 so row 0 starts from the true h0=0.
            patch[:, 6:10] = -60.0
        else:
            patch[:, 0:6] = 1.0
        in_maps.append({**common, "a2t": a2t, "xd": xd, "dinv1": dinv1,
                        "dinv2": dinv2, "dinvl": dinvl, "patch": patch})
    return in_maps


def assemble_output(results):
    outs = [r["out_t"].T for r in results]          # each [ROWS, OUT]
    full = np.concatenate(outs, axis=0).astype(np.float32)
    return full[None]                               # [1, N, OUT]


def kernel(**inputs) -> np.ndarray:
    from concourse import bass_utils

    if "nc" not in _CACHE:
        _CACHE["nc"] = build_program()
    nc = _CACHE["nc"]
    in_maps = host_prepare(inputs)
    res = bass_utils.run_bass_kernel_spmd(
        nc, in_maps, core_ids=list(range(CORES)))
    return assemble_output(res.results)


if __name__ == "__main__":
    import reference

    inputs = {k: np.asarray(v) for k, v in reference.setup_inputs().items()}
    out = kernel(**inputs)
    print("kernel out", out.shape, out.dtype)
    np.save("/root/problem/kernel_out.npy", out)
